# revision 3
# baseline (speedup 1.0000x reference)
"""Trainium2 Bass kernel for DigitConvolutionalModel forward pass.

Model: x[B,784] -> 3x3 valid conv (28x28 -> 26x26) -> flatten[676]
       -> Linear(676->200) + ReLU -> Linear(200->10).

Key algebraic optimization: the conv is linear and feeds straight into the
first Linear, so both fold into a single effective weight
W_eff[200,784] = w0 compose conv  (computed once on host, ~1.2 MFLOP).
The device then runs two dense GEMMs per batch shard:
    h = relu(x @ W_eff.T + b0);  out = h @ w1.T + b1

Sharding: pure data parallel over the batch dim across 8 NeuronCores
(4096 rows each); weights replicated; no collectives (forward only).

On-device layout is feature-major ("transposed") so the contraction dim
always lives on SBUF partitions: xT[784,n] -> hT[200,n] -> outT[10,n].
The host pre-packs x shards into exact SBUF tile images (k tiled 7x112)
so all x traffic is a handful of large single-ring DMAs whose partition
lines are multi-KB contiguous runs.

Schedule (from trace analysis of the measured exec window):
  * the PE's HAM throttle (50% util limit) runs from the framework
    preamble to ~11.9us into the trace regardless of what we do, so the
    goal is to have real matmuls streaming right at that point;
  * w0 is split by PSUM m-tile (128 + 72 hidden cols) and these two DMAs
    get the SP ring first (after the tiny bias/w1 images), while seg0
    races down the SWDGE ring concurrently -> first real matmul ~10us;
  * x segments alternate SP/SWDGE rings with completion-chained pushes
    (<=2 in flight per ring) so the SDMA engines never round-robin a
    critical segment against a later one;
  * segment widths ramp 128->512 at the head (compute starts early) and
    shrink at the tail (short relu->fc2->store latency chain);
  * per-seg output stores ride the ACT ring behind their ident; the last
    TWO segments share one store so the final (cold-latency) push is a
    single hot-ring transfer.
Compute dtype bf16 (1 cyc/row matmuls, half the DMA bytes); PSUM
accumulates f32; bias+ReLU fused on the vector engine; dummy matmuls on
zeroed scratch pre-warm the PE clock gate during the DMA flight.
"""

import os
import sys
import types
import numpy as np

for _p in ("/opt/trn_rl_repo", "/root/.axon_site"):
    if os.path.isdir(_p) and _p not in sys.path:
        sys.path.insert(0, _p)

import concourse.bass as bass  # noqa: E402
import concourse.tile as tile  # noqa: E402
import concourse.mybir as mybir  # noqa: E402
from concourse import bacc  # noqa: E402
from concourse.bass_utils import run_bass_kernel_spmd  # noqa: E402

B = 32768
N_CORES = 8
SHARD = B // N_CORES          # 4096
KDIM = 784                    # 28*28 input features (conv folded in)
HID = 200
OUT = 10
# batch-column widths per pipeline segment: ramp at the head (compute
# starts sooner, bridging the PE warm-up + DMA spin-up) and small at the
# tail (short relu->fc2->store chain after the last big matmul)
SEGS = [128, 128, 256, 256, 512, 512, 512, 512, 512, 384, 256, 128]
assert sum(SEGS) == SHARD
N_TAIL_MERGE = 2              # last segs sharing one output store
KT = 112                      # k-tile partition size (7 * 112 = 784)
NKT = KDIM // KT              # 7 k-tiles
M_TILES = [(0, 128), (128, 72)]  # hidden 200 = 128 + 72 PSUM partition tiles
N_WARMUP = 12                 # dummy matmuls to trip the HAM clock gate
WARM_W = 128                  # warm matmul free-dim width

# matmul operand dtype:
#   float32  — exact, 4 cyc/row
#   float32r — ~2-3 cyc/row, rel err 2e-4
#   bfloat16 — 1 cyc/row, half DMA bytes, rel err ~3e-3
MM_DT = mybir.dt.bfloat16

last_exec_time_ns = None      # set when BASS_KERNEL_PROFILE=1


def _install_ntff_hook():
    """Register the axon NTFF profile hook if the image's antenv lacks it."""
    try:
        from antenv.axon_hooks import get_axon_ntff_profile_hook  # noqa: F401
        return
    except ImportError:
        pass
    try:
        from trn_agent_boot.trn_boot import _ntff_profile_via_ctypes
        hook = _ntff_profile_via_ctypes("/opt/axon/libaxon_pjrt.so")
    except Exception:
        hook = None
    mod = types.ModuleType("antenv.axon_hooks")
    mod.get_axon_ntff_profile_hook = lambda: hook
    mod.set_axon_ntff_profile_hook = lambda h: None
    sys.modules["antenv.axon_hooks"] = mod


def _np_mm_dtype():
    if MM_DT == mybir.dt.bfloat16:
        import ml_dtypes
        return np.dtype(ml_dtypes.bfloat16)
    return np.dtype(np.float32)


def fold_conv_into_fc(conv_w: np.ndarray, w0: np.ndarray) -> np.ndarray:
    """W_eff[200,784] such that x @ W_eff.T == fc1(flatten(conv(x)))."""
    w0v = w0.reshape(HID, 26, 26).astype(np.float64)
    w_img = np.zeros((HID, 28, 28), dtype=np.float64)
    for ki in range(3):
        for kj in range(3):
            w_img[:, ki:ki + 26, kj:kj + 26] += w0v * np.float64(conv_w[ki, kj])
    return w_img.reshape(HID, KDIM).astype(np.float32)


def pack_shard(xs: np.ndarray, mm_np):
    """Pack one x shard [4096, 784] into per-segment SBUF tile images.

    Segment g (w cols starting at batch row c0):
      xg[p, a, n] = x[c0 + n, a*KT + p]
    Every SBUF partition line is one contiguous (a, n) run.
    """
    xsv = xs.reshape(SHARD, NKT, KT)
    arrays = []
    c0 = 0
    for w in SEGS:
        blk = xsv[c0:c0 + w]                        # [n, a, p]
        arrays.append(np.ascontiguousarray(
            blk.transpose(2, 1, 0).astype(mm_np)))  # [p, a, n]
        c0 += w
    return arrays


def pack_weights(w_eff: np.ndarray, w1: np.ndarray, b0, b1, mm_np):
    """Pack weights/biases into single-DMA SBUF images.

    w0 is split by PSUM m-tile so the 128-col half can land (and the
    first matmuls start) before the 72-col half finishes its DMA.
    """
    # w0sb[p, a, m] = W_eff[m, a*KT + p], split m into 0:128 / 128:200
    w0sb = w_eff.reshape(HID, NKT, KT).transpose(2, 1, 0).astype(mm_np)
    w0a = np.ascontiguousarray(w0sb[:, :, 0:128])
    w0b = np.ascontiguousarray(w0sb[:, :, 128:HID])
    # w1sb[p, 0:10] = w1[:, p].T ; w1sb[0:72, 10:20] = w1[:, 128+p].T
    w1sb = np.zeros((128, 2 * OUT), dtype=mm_np)
    w1sb[:, :OUT] = w1[:, 0:128].T.astype(mm_np)
    w1sb[:HID - 128, OUT:] = w1[:, 128:HID].T.astype(mm_np)
    # bias[p, 0] = b0[p]; bias[0:72, 1] = b0[128:200]; bias[0:10, 2] = b1
    biases = np.zeros((128, 3), dtype=np.float32)
    biases[:, 0] = b0[0:128]
    biases[:HID - 128, 1] = b0[128:HID]
    biases[:OUT, 2] = b1
    return w0a, w0b, w1sb, biases


def build_program():
    nc = bacc.Bacc("TRN2", target_bir_lowering=False, debug=False)
    f32 = mybir.dt.float32
    add = mybir.AluOpType.add
    amax = mybir.AluOpType.max

    xg_d = [
        nc.declare_dram_parameter(
            f"xg{g}", [KT, NKT, w], MM_DT, isOutput=False)
        for g, w in enumerate(SEGS)
    ]
    w0a_d = nc.declare_dram_parameter("w0a", [KT, NKT, 128], MM_DT, isOutput=False)
    w0b_d = nc.declare_dram_parameter("w0b", [KT, NKT, HID - 128], MM_DT,
                                      isOutput=False)
    w1_d = nc.declare_dram_parameter("w1sb", [128, 2 * OUT], MM_DT, isOutput=False)
    bia_d = nc.declare_dram_parameter("biases", [128, 3], f32, isOutput=False)
    out_d = nc.declare_dram_parameter("out", [OUT, SHARD], f32, isOutput=True)

    n_segs = len(SEGS)
    tail0 = n_segs - N_TAIL_MERGE          # first seg of the merged store
    tail_w = sum(SEGS[tail0:])
    tail_c0 = SHARD - tail_w

    with tile.TileContext(nc) as tc:
        with (
            tc.tile_pool(name="weights", bufs=1) as wpool,
            tc.tile_pool(name="xin", bufs=n_segs) as xpool,
            tc.tile_pool(name="hbuf", bufs=2) as hpool,
            tc.tile_pool(name="obuf", bufs=4) as opool,
            tc.tile_pool(name="psum", bufs=2, space=bass.MemorySpace.PSUM) as pp,
            tc.tile_pool(name="opsum", bufs=2, space=bass.MemorySpace.PSUM) as op,
        ):
            # tiny bias/w1 images lead the SP ring (~7KB, no measurable
            # delay to w0), then the two w0 halves; seg0 races down the
            # SWDGE ring concurrently. The ACT ring is kept clear for
            # the per-segment output stores.
            bia = wpool.tile([128, 3], f32)
            nc.sync.dma_start(bia[:], bia_d[:])
            w1 = wpool.tile([128, 2 * OUT], MM_DT)
            nc.sync.dma_start(w1[:], w1_d[:])
            w0t = [wpool.tile([KT, NKT, dm], MM_DT, name=f"w0_{mi}")
                   for mi, (m0, dm) in enumerate(M_TILES)]
            w0_dmas = [nc.sync.dma_start(w0t[mi][:], d[:])
                       for mi, d in enumerate((w0a_d, w0b_d))]

            # PE pre-warm on zeroed scratch while the first DMAs fly.
            # memset rides the (otherwise idle at t0) vector queue so the
            # SWDGE queue's first instruction is seg0's push.
            warm_x = wpool.tile([KT, WARM_W], MM_DT)
            nc.vector.memset(warm_x[:], 0.0)
            warm_ps = op.tile([128, WARM_W], f32, tag="warm", bufs=1)
            for _ in range(N_WARMUP):
                nc.tensor.matmul(
                    warm_ps[:], warm_x[:, 0:128], warm_x[:],
                    start=True, stop=True)

            o_tail = opool.tile([OUT, tail_w], f32, tag="osb_tail", bufs=1)

            def emit_layer2(g, w, c0, h_tiles):
                # layer 2: outT[10, seg], 2 accumulating matmuls
                o_ps = op.tile([OUT, w], f32, tag="ops", name=f"ops_{g}")
                nc.tensor.matmul(
                    o_ps[:], w1[0:128, 0:OUT], h_tiles[0][:],
                    start=True, stop=False)
                nc.tensor.matmul(
                    o_ps[:], w1[0:HID - 128, OUT:2 * OUT], h_tiles[1][:],
                    start=False, stop=True)
                # bias-add on the scalar engine: runs in PARALLEL with the
                # DVE's relu of the next segment. The last N_TAIL_MERGE
                # segments write slices of one SBUF tile (allocated once,
                # so RAW deps track the partial writes) and share one
                # store: the final push is a single transfer on the
                # already-hot ACT ring.
                if g >= tail0:
                    lo = c0 - tail_c0
                    nc.scalar.activation(
                        o_tail[:, lo:lo + w], o_ps[:],
                        mybir.ActivationFunctionType.Identity,
                        bias=bia[0:OUT, 2:3])
                    if g == n_segs - 1:
                        nc.scalar.dma_start(
                            out_d[:, tail_c0:SHARD], o_tail[:])
                else:
                    o_sb = opool.tile([OUT, w], f32, tag="osb",
                                      name=f"osb_{g}")
                    nc.scalar.activation(
                        o_sb[:], o_ps[:],
                        mybir.ActivationFunctionType.Identity,
                        bias=bia[0:OUT, 2:3])
                    nc.scalar.dma_start(out_d[:, c0:c0 + w], o_sb[:])

            c0 = 0
            ring_dmas = {0: [], 1: []}   # SWDGE / SP ring push history
            pending = None   # layer 2 runs one segment behind layer 1,
            # so the PE never waits on the DVE relu at a seg boundary
            for g, w in enumerate(SEGS):
                xg = xpool.tile([KT, NKT, w], MM_DT, tag="xg",
                                name=f"xg_{g}")
                # alternate rings: even segs SWDGE (gpsimd), odd SP (sync)
                ring = g % 2
                eng = nc.sync if ring else nc.gpsimd
                dma = eng.dma_start(xg[:], xg_d[g][:])
                hist = ring_dmas[ring]
                if ring == 1 and not hist:
                    # seg1 queues behind the critical w0a so the SDMA
                    # engines don't round-robin it against both w0 halves
                    tile.add_dep_helper(
                        dma.ins, w0_dmas[0].ins, sync=True,
                        reason="keep SP ring focused on w0 until it lands")
                elif hist:
                    # chain same-ring pushes on the previous same-ring
                    # completion: <=2 transfers in flight per ring, so a
                    # segment the PE needs next never round-robins
                    # against later ones
                    tile.add_dep_helper(
                        dma.ins, hist[-1].ins, sync=True,
                        reason="throttle per-ring x DMA depth")
                hist.append(dma)

                # layer 1: hT[m0:m0+dm, seg], 7 accumulating matmuls per
                # m-tile; m-tile 0 only needs the w0a image so it can
                # start while w0b is still in flight
                h_tiles = []
                for mi, (m0, dm) in enumerate(M_TILES):
                    h_ps = pp.tile([dm, w], f32, tag=f"hps{mi}",
                                   name=f"hps_{g}_{mi}")
                    for a in range(NKT):
                        nc.tensor.matmul(
                            h_ps[:],
                            w0t[mi][:, a, :],
                            xg[:, a, :],
                            start=(a == 0),
                            stop=(a == NKT - 1),
                        )
                    h_sb = hpool.tile([dm, w], MM_DT, tag=f"h{mi}",
                                      name=f"h_{g}_{mi}")
                    # fused bias + relu on the vector engine
                    nc.vector.tensor_scalar(
                        h_sb[:], h_ps[:], bia[0:dm, mi:mi + 1], 0.0,
                        add, amax)
                    h_tiles.append(h_sb)

                if pending is not None:
                    emit_layer2(*pending)
                pending = (g, w, c0, h_tiles)
                c0 += w

            emit_layer2(*pending)

    nc.compile()
    return nc


_program_cache = {}


def _get_program():
    key = (MM_DT, tuple(SEGS), N_WARMUP)
    if key not in _program_cache:
        _program_cache[key] = build_program()
    return _program_cache[key]


def kernel(**inputs: np.ndarray) -> np.ndarray:
    x = np.asarray(inputs["x"], dtype=np.float32)
    conv_w = np.asarray(inputs["conv_w"], dtype=np.float32)
    w0 = np.asarray(inputs["w0"], dtype=np.float32)
    b0 = np.asarray(inputs["b0"], dtype=np.float32)
    w1 = np.asarray(inputs["w1"], dtype=np.float32)
    b1 = np.asarray(inputs["b1"], dtype=np.float32)

    mm_np = _np_mm_dtype()
    w_eff = fold_conv_into_fc(conv_w, w0)
    w0a, w0b, w1sb, biases = pack_weights(w_eff, w1, b0, b1, mm_np)

    in_maps = []
    for i in range(N_CORES):
        xgs = pack_shard(x[i * SHARD:(i + 1) * SHARD], mm_np)
        m = {f"xg{g}": xg for g, xg in enumerate(xgs)}
        m.update({"w0a": w0a, "w0b": w0b, "w1sb": w1sb, "biases": biases})
        in_maps.append(m)

    nc = _get_program()

    profile = os.environ.get("BASS_KERNEL_PROFILE", "0") == "1"
    kwargs = {}
    if profile:
        _install_ntff_hook()
        kwargs = dict(trace=True, tmpdir=os.environ.get("BASS_KERNEL_TRACE_DIR"))
    try:
        res = run_bass_kernel_spmd(
            nc, in_maps, core_ids=list(range(N_CORES)), **kwargs)
    except Exception:
        # a previous process can leave a NeuronCore momentarily
        # unrecoverable (NRT_EXEC_UNIT_UNRECOVERABLE); one retry suffices
        import time
        time.sleep(5)
        res = run_bass_kernel_spmd(
            nc, in_maps, core_ids=list(range(N_CORES)), **kwargs)

    global last_exec_time_ns
    last_exec_time_ns = res.exec_time_ns

    out = np.empty((B, OUT), dtype=np.float32)
    for i in range(N_CORES):
        out[i * SHARD:(i + 1) * SHARD] = res.results[i]["out"].T
    return out


# revision 6
# speedup vs baseline: 1.0164x; 1.0164x over previous
"""Trainium2 Bass kernel for DigitConvolutionalModel forward pass.

Model: x[B,784] -> 3x3 valid conv (28x28 -> 26x26) -> flatten[676]
       -> Linear(676->200) + ReLU -> Linear(200->10).

Key algebraic optimization: the conv is linear and feeds straight into the
first Linear, so both fold into a single effective weight
W_eff[200,784] = w0 compose conv  (computed once on host, ~1.2 MFLOP).
The device then runs two dense GEMMs per batch shard:
    h = relu(x @ W_eff.T + b0);  out = h @ w1.T + b1

Sharding: pure data parallel over the batch dim across 8 NeuronCores
(4096 rows each); weights replicated; no collectives (forward only).

On-device layout is feature-major ("transposed") so the contraction dim
always lives on SBUF partitions: xT[784,n] -> hT[200,n] -> outT[10,n].
The host pre-packs x shards into exact SBUF tile images (k tiled 7x112)
so all x traffic is a handful of large single-ring DMAs whose partition
lines are multi-KB contiguous runs.

Schedule (from trace analysis of the measured exec window):
  * the PE's HAM throttle (50% util limit) runs from the framework
    preamble to ~11.9us into the trace regardless of what we do, so the
    goal is to have real matmuls streaming right at that point;
  * w0 is split by PSUM m-tile (128 + 72 hidden cols) and these two DMAs
    get the SP ring first (after the tiny bias/w1 images), while seg0
    races down the SWDGE ring concurrently -> first real matmul ~10us;
  * x segments alternate SP/SWDGE rings with completion-chained pushes
    (<=2 in flight per ring) so the SDMA engines never round-robin a
    critical segment against a later one;
  * segment widths ramp 128->512 at the head (compute starts early) and
    shrink at the tail (short relu->fc2->store latency chain);
  * per-seg output stores ride the ACT ring behind their ident; the last
    TWO segments share one store so the final (cold-latency) push is a
    single hot-ring transfer.
Compute dtype bf16 (1 cyc/row matmuls, half the DMA bytes); PSUM
accumulates f32; bias+ReLU fused on the vector engine; dummy matmuls on
zeroed scratch pre-warm the PE clock gate during the DMA flight.
"""

import os
import sys
import types
import numpy as np

for _p in ("/opt/trn_rl_repo", "/root/.axon_site"):
    if os.path.isdir(_p) and _p not in sys.path:
        sys.path.insert(0, _p)

import concourse.bass as bass  # noqa: E402
import concourse.tile as tile  # noqa: E402
import concourse.mybir as mybir  # noqa: E402
from concourse import bacc  # noqa: E402
from concourse.bass_utils import run_bass_kernel_spmd  # noqa: E402

B = 32768
N_CORES = 8
SHARD = B // N_CORES          # 4096
KDIM = 784                    # 28*28 input features (conv folded in)
HID = 200
OUT = 10
# batch-column widths per pipeline segment: ramp at the head (compute
# starts sooner, bridging the PE warm-up + DMA spin-up) and small at the
# tail (short relu->fc2->store chain after the last big matmul)
SEGS = [128, 128, 256, 256, 512, 512, 512, 512, 512, 384, 256, 128]
assert sum(SEGS) == SHARD
N_TAIL_MERGE = 2              # last segs sharing one output store
KT = 112                      # k-tile partition size (7 * 112 = 784)
NKT = KDIM // KT              # 7 k-tiles
M_TILES = [(0, 128), (128, 72)]  # hidden 200 = 128 + 72 PSUM partition tiles
N_WARMUP = 20                 # dummy matmuls to trip the HAM clock gate
WARM_W = 128                  # warm matmul free-dim width

# matmul operand dtype:
#   float32  — exact, 4 cyc/row
#   float32r — ~2-3 cyc/row, rel err 2e-4
#   bfloat16 — 1 cyc/row, half DMA bytes, rel err ~3e-3
MM_DT = mybir.dt.bfloat16

last_exec_time_ns = None      # set when BASS_KERNEL_PROFILE=1


def _install_ntff_hook():
    """Register the axon NTFF profile hook if the image's antenv lacks it."""
    try:
        from antenv.axon_hooks import get_axon_ntff_profile_hook  # noqa: F401
        return
    except ImportError:
        pass
    try:
        from trn_agent_boot.trn_boot import _ntff_profile_via_ctypes
        hook = _ntff_profile_via_ctypes("/opt/axon/libaxon_pjrt.so")
    except Exception:
        hook = None
    mod = types.ModuleType("antenv.axon_hooks")
    mod.get_axon_ntff_profile_hook = lambda: hook
    mod.set_axon_ntff_profile_hook = lambda h: None
    sys.modules["antenv.axon_hooks"] = mod


def _np_mm_dtype():
    if MM_DT == mybir.dt.bfloat16:
        import ml_dtypes
        return np.dtype(ml_dtypes.bfloat16)
    return np.dtype(np.float32)


def fold_conv_into_fc(conv_w: np.ndarray, w0: np.ndarray) -> np.ndarray:
    """W_eff[200,784] such that x @ W_eff.T == fc1(flatten(conv(x)))."""
    w0v = w0.reshape(HID, 26, 26).astype(np.float64)
    w_img = np.zeros((HID, 28, 28), dtype=np.float64)
    for ki in range(3):
        for kj in range(3):
            w_img[:, ki:ki + 26, kj:kj + 26] += w0v * np.float64(conv_w[ki, kj])
    return w_img.reshape(HID, KDIM).astype(np.float32)


def pack_shard(xs: np.ndarray, mm_np):
    """Pack one x shard [4096, 784] into per-segment SBUF tile images.

    Segment g (w cols starting at batch row c0):
      xg[p, a, n] = x[c0 + n, a*KT + p]
    Every SBUF partition line is one contiguous (a, n) run.
    """
    xsv = xs.reshape(SHARD, NKT, KT)
    arrays = []
    c0 = 0
    for w in SEGS:
        blk = xsv[c0:c0 + w]                        # [n, a, p]
        arrays.append(np.ascontiguousarray(
            blk.transpose(2, 1, 0).astype(mm_np)))  # [p, a, n]
        c0 += w
    return arrays


def pack_weights(w_eff: np.ndarray, w1: np.ndarray, b0, b1, mm_np):
    """Pack weights/biases into single-DMA SBUF images.

    w0 is split by PSUM m-tile so the 128-col half can land (and the
    first matmuls start) before the 72-col half finishes its DMA.
    """
    # w0sb[p, a, m] = W_eff[m, a*KT + p], split m into 0:128 / 128:200
    w0sb = w_eff.reshape(HID, NKT, KT).transpose(2, 1, 0).astype(mm_np)
    w0a = np.ascontiguousarray(w0sb[:, :, 0:128])
    w0b = np.ascontiguousarray(w0sb[:, :, 128:HID])
    # w1sb[p, 0:10] = w1[:, p].T ; w1sb[0:72, 10:20] = w1[:, 128+p].T
    w1sb = np.zeros((128, 2 * OUT), dtype=mm_np)
    w1sb[:, :OUT] = w1[:, 0:128].T.astype(mm_np)
    w1sb[:HID - 128, OUT:] = w1[:, 128:HID].T.astype(mm_np)
    # bias[p, 0] = b0[p]; bias[0:72, 1] = b0[128:200]; bias[0:10, 2] = b1
    biases = np.zeros((128, 3), dtype=np.float32)
    biases[:, 0] = b0[0:128]
    biases[:HID - 128, 1] = b0[128:HID]
    biases[:OUT, 2] = b1
    return w0a, w0b, w1sb, biases


def build_program():
    nc = bacc.Bacc("TRN2", target_bir_lowering=False, debug=False)
    f32 = mybir.dt.float32
    add = mybir.AluOpType.add
    amax = mybir.AluOpType.max

    xg_d = [
        nc.declare_dram_parameter(
            f"xg{g}", [KT, NKT, w], MM_DT, isOutput=False)
        for g, w in enumerate(SEGS)
    ]
    w0a_d = nc.declare_dram_parameter("w0a", [KT, NKT, 128], MM_DT, isOutput=False)
    w0b_d = nc.declare_dram_parameter("w0b", [KT, NKT, HID - 128], MM_DT,
                                      isOutput=False)
    w1_d = nc.declare_dram_parameter("w1sb", [128, 2 * OUT], MM_DT, isOutput=False)
    bia_d = nc.declare_dram_parameter("biases", [128, 3], f32, isOutput=False)
    out_d = nc.declare_dram_parameter("out", [OUT, SHARD], f32, isOutput=True)

    n_segs = len(SEGS)
    tail0 = n_segs - N_TAIL_MERGE          # first seg of the merged store
    tail_w = sum(SEGS[tail0:])
    tail_c0 = SHARD - tail_w

    with tile.TileContext(nc) as tc:
        with (
            tc.tile_pool(name="weights", bufs=1) as wpool,
            tc.tile_pool(name="xin", bufs=n_segs) as xpool,
            tc.tile_pool(name="hbuf", bufs=2) as hpool,
            tc.tile_pool(name="obuf", bufs=4) as opool,
            tc.tile_pool(name="psum", bufs=2, space=bass.MemorySpace.PSUM) as pp,
            tc.tile_pool(name="opsum", bufs=2, space=bass.MemorySpace.PSUM) as op,
        ):
            # tiny bias/w1 images lead the SP ring (~7KB, no measurable
            # delay to w0), then the two w0 halves; seg0 races down the
            # SWDGE ring concurrently. The ACT ring is kept clear for
            # the per-segment output stores.
            bia = wpool.tile([128, 3], f32)
            nc.sync.dma_start(bia[:], bia_d[:])
            w1 = wpool.tile([128, 2 * OUT], MM_DT)
            nc.sync.dma_start(w1[:], w1_d[:])
            w0t = [wpool.tile([KT, NKT, dm], MM_DT, name=f"w0_{mi}")
                   for mi, (m0, dm) in enumerate(M_TILES)]
            w0_dmas = [nc.sync.dma_start(w0t[mi][:], d[:])
                       for mi, d in enumerate((w0a_d, w0b_d))]

            # PE pre-warm on zeroed scratch while the first DMAs fly.
            # memset rides the (otherwise idle at t0) vector queue so the
            # SWDGE queue's first instruction is seg0's push.
            warm_x = wpool.tile([KT, WARM_W], MM_DT)
            nc.vector.memset(warm_x[:], 0.0)
            warm_ps = op.tile([128, WARM_W], f32, tag="warm", bufs=1)
            for _ in range(N_WARMUP):
                nc.tensor.matmul(
                    warm_ps[:], warm_x[:, 0:128], warm_x[:],
                    start=True, stop=True)

            o_tail = opool.tile([OUT, tail_w], f32, tag="osb_tail", bufs=1)

            def emit_layer2(g, w, c0, h_tiles):
                # layer 2: outT[10, seg], 2 accumulating matmuls
                o_ps = op.tile([OUT, w], f32, tag="ops", name=f"ops_{g}")
                nc.tensor.matmul(
                    o_ps[:], w1[0:128, 0:OUT], h_tiles[0][:],
                    start=True, stop=False)
                nc.tensor.matmul(
                    o_ps[:], w1[0:HID - 128, OUT:2 * OUT], h_tiles[1][:],
                    start=False, stop=True)
                # bias-add on the scalar engine: runs in PARALLEL with the
                # DVE's relu of the next segment. The last N_TAIL_MERGE
                # segments write slices of one SBUF tile (allocated once,
                # so RAW deps track the partial writes) and share one
                # store: the final push is a single transfer on the
                # already-hot ACT ring.
                if g >= tail0:
                    lo = c0 - tail_c0
                    nc.scalar.activation(
                        o_tail[:, lo:lo + w], o_ps[:],
                        mybir.ActivationFunctionType.Identity,
                        bias=bia[0:OUT, 2:3])
                    if g == n_segs - 1:
                        nc.scalar.dma_start(
                            out_d[:, tail_c0:SHARD], o_tail[:])
                else:
                    o_sb = opool.tile([OUT, w], f32, tag="osb",
                                      name=f"osb_{g}")
                    nc.scalar.activation(
                        o_sb[:], o_ps[:],
                        mybir.ActivationFunctionType.Identity,
                        bias=bia[0:OUT, 2:3])
                    nc.scalar.dma_start(out_d[:, c0:c0 + w], o_sb[:])

            # x segment -> DMA ring. The head is spread across all three
            # rings (seg0 solo on SWDGE, w0a/w0b lead SP, seg2 leads ACT)
            # because DMA bandwidth is itself ramping until ~12us; after
            # that each ring keeps a rolling depth-2 queue (push gated on
            # the 2-back same-ring completion) so rings never idle
            # between transfers but a critical segment never round-robins
            # against more than one later transfer.
            seg_eng = {0: nc.gpsimd, 1: nc.sync, 2: nc.scalar,
                       3: nc.sync, 4: nc.gpsimd, 5: nc.sync,
                       6: nc.gpsimd, 7: nc.sync, 8: nc.gpsimd,
                       9: nc.sync, 10: nc.gpsimd, 11: nc.sync}
            # push of seg g waits for this prior transfer's completion:
            seg_dep = {1: "w0a", 3: "w0b", 5: 1, 7: 3, 9: 5, 11: 7,
                       4: 0, 6: 0, 8: 4, 10: 6}

            # all x pushes are hoisted ahead of the compute emission so
            # every ring's queue starts with its transfers (in particular
            # seg2's push must not sit behind the first ident on the ACT
            # queue)
            xg_tiles = []
            seg_dmas = {}
            for g, w in enumerate(SEGS):
                xg = xpool.tile([KT, NKT, w], MM_DT, tag="xg",
                                name=f"xg_{g}")
                xg_tiles.append(xg)
                dma = seg_eng[g].dma_start(xg[:], xg_d[g][:])
                dep = seg_dep.get(g)
                if dep == "w0a":
                    tile.add_dep_helper(
                        dma.ins, w0_dmas[0].ins, sync=True,
                        reason="SP ring: x behind critical w0a")
                elif dep == "w0b":
                    tile.add_dep_helper(
                        dma.ins, w0_dmas[1].ins, sync=True,
                        reason="SP ring: rolling depth-2")
                elif dep is not None:
                    tile.add_dep_helper(
                        dma.ins, seg_dmas[dep].ins, sync=True,
                        reason="rolling depth-2 per-ring x DMA queue")
                seg_dmas[g] = dma

            c0 = 0
            pending = None   # layer 2 runs one segment behind layer 1,
            # so the PE never waits on the DVE relu at a seg boundary
            for g, w in enumerate(SEGS):
                xg = xg_tiles[g]

                # layer 1: hT[m0:m0+dm, seg], 7 accumulating matmuls per
                # m-tile; m-tile 0 only needs the w0a image so it can
                # start while w0b is still in flight
                h_tiles = []
                for mi, (m0, dm) in enumerate(M_TILES):
                    h_ps = pp.tile([dm, w], f32, tag=f"hps{mi}",
                                   name=f"hps_{g}_{mi}")
                    for a in range(NKT):
                        nc.tensor.matmul(
                            h_ps[:],
                            w0t[mi][:, a, :],
                            xg[:, a, :],
                            start=(a == 0),
                            stop=(a == NKT - 1),
                        )
                    h_sb = hpool.tile([dm, w], MM_DT, tag=f"h{mi}",
                                      name=f"h_{g}_{mi}")
                    # fused bias + relu on the vector engine
                    nc.vector.tensor_scalar(
                        h_sb[:], h_ps[:], bia[0:dm, mi:mi + 1], 0.0,
                        add, amax)
                    h_tiles.append(h_sb)

                if pending is not None:
                    emit_layer2(*pending)
                pending = (g, w, c0, h_tiles)
                c0 += w

            emit_layer2(*pending)

    nc.compile()
    return nc


_program_cache = {}


def _get_program():
    key = (MM_DT, tuple(SEGS), N_WARMUP)
    if key not in _program_cache:
        _program_cache[key] = build_program()
    return _program_cache[key]


def kernel(**inputs: np.ndarray) -> np.ndarray:
    x = np.asarray(inputs["x"], dtype=np.float32)
    conv_w = np.asarray(inputs["conv_w"], dtype=np.float32)
    w0 = np.asarray(inputs["w0"], dtype=np.float32)
    b0 = np.asarray(inputs["b0"], dtype=np.float32)
    w1 = np.asarray(inputs["w1"], dtype=np.float32)
    b1 = np.asarray(inputs["b1"], dtype=np.float32)

    mm_np = _np_mm_dtype()
    w_eff = fold_conv_into_fc(conv_w, w0)
    w0a, w0b, w1sb, biases = pack_weights(w_eff, w1, b0, b1, mm_np)

    in_maps = []
    for i in range(N_CORES):
        xgs = pack_shard(x[i * SHARD:(i + 1) * SHARD], mm_np)
        m = {f"xg{g}": xg for g, xg in enumerate(xgs)}
        m.update({"w0a": w0a, "w0b": w0b, "w1sb": w1sb, "biases": biases})
        in_maps.append(m)

    nc = _get_program()

    profile = os.environ.get("BASS_KERNEL_PROFILE", "0") == "1"
    kwargs = {}
    if profile:
        _install_ntff_hook()
        kwargs = dict(trace=True, tmpdir=os.environ.get("BASS_KERNEL_TRACE_DIR"))
    try:
        res = run_bass_kernel_spmd(
            nc, in_maps, core_ids=list(range(N_CORES)), **kwargs)
    except Exception:
        # a previous process can leave a NeuronCore momentarily
        # unrecoverable (NRT_EXEC_UNIT_UNRECOVERABLE); one retry suffices
        import time
        time.sleep(5)
        res = run_bass_kernel_spmd(
            nc, in_maps, core_ids=list(range(N_CORES)), **kwargs)

    global last_exec_time_ns
    last_exec_time_ns = res.exec_time_ns

    out = np.empty((B, OUT), dtype=np.float32)
    for i in range(N_CORES):
        out[i * SHARD:(i + 1) * SHARD] = res.results[i]["out"].T
    return out


# revision 12
# speedup vs baseline: 1.0748x; 1.0575x over previous
"""Trainium2 Bass kernel for DigitConvolutionalModel forward pass.

Model: x[B,784] -> 3x3 valid conv (28x28 -> 26x26) -> flatten[676]
       -> Linear(676->200) + ReLU -> Linear(200->10).

Key algebraic optimization: the conv is linear and feeds straight into the
first Linear, so both fold into a single effective weight
W_eff[200,784] = w0 compose conv  (computed once on host, ~1.2 MFLOP).
The device then runs two dense GEMMs per batch shard:
    h = relu(x @ W_eff.T + b0);  out = h @ w1.T + b1

Sharding: pure data parallel over the batch dim across 8 NeuronCores
(4096 rows each); weights replicated; no collectives (forward only).

On-device layout is feature-major ("transposed") so the contraction dim
always lives on SBUF partitions: xT[784,n] -> hT[200,n] -> outT[10,n].
The host pre-packs x shards into exact SBUF tile images (k tiled 7x112)
so all x traffic is a handful of large single-ring DMAs whose partition
lines are multi-KB contiguous runs; group sizes are staggered (small
first) so compute starts early. Compute dtype bf16 (1 cyc/row matmuls,
half the DMA bytes); PSUM accumulates f32; bias+ReLU fused on the
vector engine; weights load on the ACT ring; output stores on SWDGE.
Dummy matmuls on zeroed scratch pre-warm the PE's HAM clock gate during
the first DMA's flight.
"""

import os
import sys
import types
import numpy as np

for _p in ("/opt/trn_rl_repo", "/root/.axon_site"):
    if os.path.isdir(_p) and _p not in sys.path:
        sys.path.insert(0, _p)

import concourse.bass as bass  # noqa: E402
import concourse.tile as tile  # noqa: E402
import concourse.mybir as mybir  # noqa: E402
from concourse import bacc  # noqa: E402
from concourse.bass_utils import run_bass_kernel_spmd  # noqa: E402

B = 32768
N_CORES = 8
SHARD = B // N_CORES          # 4096
KDIM = 784                    # 28*28 input features (conv folded in)
HID = 200
OUT = 10
CHUNK = 512                   # batch columns per matmul (moving free dim)
# batch-column widths per pipeline segment: narrow at the head (compute
# starts sooner, bridging the PE warm-up) and at the tail (shorter
# relu->fc2->store latency chain after the last big matmul); the last
# two segments share one output store
SEGS = [256] + [512] * 7 + [128, 128]
N_TAIL_MERGE = 2
KT = 112                      # k-tile partition size (7 * 112 = 784)
NKT = KDIM // KT              # 7 k-tiles
M_TILES = [(0, 128), (128, 72)]  # hidden 200 = 128 + 72 PSUM partition tiles
N_WARMUP = 11                 # dummy matmuls to trip the HAM clock gate

# matmul operand dtype:
#   float32  — exact, 4 cyc/row           (~143us)
#   float32r — ~2-3 cyc/row, rel err 2e-4 (~71us)
#   bfloat16 — 1 cyc/row, half DMA bytes, rel err ~3e-3
MM_DT = mybir.dt.bfloat16

last_exec_time_ns = None      # set when BASS_KERNEL_PROFILE=1


def _install_ntff_hook():
    """Register the axon NTFF profile hook if the image's antenv lacks it."""
    try:
        from antenv.axon_hooks import get_axon_ntff_profile_hook  # noqa: F401
        return
    except ImportError:
        pass
    try:
        from trn_agent_boot.trn_boot import _ntff_profile_via_ctypes
        hook = _ntff_profile_via_ctypes("/opt/axon/libaxon_pjrt.so")
    except Exception:
        hook = None
    mod = types.ModuleType("antenv.axon_hooks")
    mod.get_axon_ntff_profile_hook = lambda: hook
    mod.set_axon_ntff_profile_hook = lambda h: None
    sys.modules["antenv.axon_hooks"] = mod


def _np_mm_dtype():
    if MM_DT == mybir.dt.bfloat16:
        import ml_dtypes
        return np.dtype(ml_dtypes.bfloat16)
    return np.dtype(np.float32)


def fold_conv_into_fc(conv_w: np.ndarray, w0: np.ndarray) -> np.ndarray:
    """W_eff[200,784] such that x @ W_eff.T == fc1(flatten(conv(x)))."""
    w0v = w0.reshape(HID, 26, 26).astype(np.float64)
    w_img = np.zeros((HID, 28, 28), dtype=np.float64)
    for ki in range(3):
        for kj in range(3):
            w_img[:, ki:ki + 26, kj:kj + 26] += w0v * np.float64(conv_w[ki, kj])
    return w_img.reshape(HID, KDIM).astype(np.float32)


def pack_shard(xs: np.ndarray, mm_np):
    """Pack one x shard [4096, 784] into per-group SBUF tile images.

    Group g (gsz chunks starting at chunk c0):
      xg[p, j, a, n] = x[(c0+j)*CHUNK + n, a*KT + p]
    Every SBUF partition line is one contiguous (j, a, n) run.
    """
    xsv = xs.reshape(SHARD, NKT, KT)
    arrays = []
    c0 = 0
    for w in SEGS:
        blk = xsv[c0:c0 + w]                        # [n, a, p]
        arrays.append(np.ascontiguousarray(
            blk.transpose(2, 1, 0).astype(mm_np)))  # [p, a, n]
        c0 += w
    return arrays


def pack_weights(w_eff: np.ndarray, w1: np.ndarray, b0, b1, mm_np):
    """Pack weights/biases into single-DMA SBUF images.

    w0 is split by PSUM m-tile so the 128-col half can land (and the
    first matmuls start) slightly before the 72-col half's DMA signals.
    """
    # w0sb[p, a, m] = W_eff[m, a*KT + p], split m into 0:128 / 128:200
    w0sb = w_eff.reshape(HID, NKT, KT).transpose(2, 1, 0).astype(mm_np)
    w0a = np.ascontiguousarray(w0sb[:, :, 0:128])
    w0b = np.ascontiguousarray(w0sb[:, :, 128:HID])
    # w1sb[p, 0:10] = w1[:, p].T ; w1sb[0:72, 10:20] = w1[:, 128+p].T
    w1sb = np.zeros((128, 2 * OUT), dtype=mm_np)
    w1sb[:, :OUT] = w1[:, 0:128].T.astype(mm_np)
    w1sb[:HID - 128, OUT:] = w1[:, 128:HID].T.astype(mm_np)
    # bias[p, 0] = b0[p]; bias[0:72, 1] = b0[128:200]; bias[0:10, 2] = b1
    biases = np.zeros((128, 3), dtype=np.float32)
    biases[:, 0] = b0[0:128]
    biases[:HID - 128, 1] = b0[128:HID]
    biases[:OUT, 2] = b1
    return w0a, w0b, w1sb, biases


def build_program():
    nc = bacc.Bacc("TRN2", target_bir_lowering=False, debug=False)
    f32 = mybir.dt.float32
    add = mybir.AluOpType.add
    amax = mybir.AluOpType.max

    xg_d = [
        nc.declare_dram_parameter(
            f"xg{g}", [KT, NKT, w], MM_DT, isOutput=False)
        for g, w in enumerate(SEGS)
    ]
    w0a_d = nc.declare_dram_parameter("w0a", [KT, NKT, 128], MM_DT, isOutput=False)
    w0b_d = nc.declare_dram_parameter("w0b", [KT, NKT, HID - 128], MM_DT,
                                      isOutput=False)
    w1_d = nc.declare_dram_parameter("w1sb", [128, 2 * OUT], MM_DT, isOutput=False)
    bia_d = nc.declare_dram_parameter("biases", [128, 3], f32, isOutput=False)
    out_d = nc.declare_dram_parameter("out", [OUT, SHARD], f32, isOutput=True)

    n_segs = len(SEGS)
    tail0 = n_segs - N_TAIL_MERGE          # first seg of the merged store
    tail_w = sum(SEGS[tail0:])
    tail_c0 = SHARD - tail_w

    with tile.TileContext(nc) as tc:
        with (
            tc.tile_pool(name="weights", bufs=1) as wpool,
            tc.tile_pool(name="xin", bufs=4) as xpool,
            tc.tile_pool(name="hbuf", bufs=2) as hpool,
            tc.tile_pool(name="obuf", bufs=4) as opool,
            tc.tile_pool(name="psum", bufs=2, space=bass.MemorySpace.PSUM) as pp,
            tc.tile_pool(name="opsum", bufs=2, space=bass.MemorySpace.PSUM) as op,
        ):
            # weights + biases ride the ACT ring so the SP/SWDGE rings
            # belong exclusively to the x stream (first-chunk completion
            # time). Order: tiny bias first (needed by the first relu,
            # and it must not queue behind both w0 halves), then w0a
            # (gates the first matmul), w0b, w1.
            bia = wpool.tile([128, 3], f32)
            nc.scalar.dma_start(bia[:], bia_d[:])
            w0t = [wpool.tile([KT, NKT, dm], MM_DT, name=f"w0_{mi}")
                   for mi, (m0, dm) in enumerate(M_TILES)]
            for mi, d in enumerate((w0a_d, w0b_d)):
                nc.scalar.dma_start(w0t[mi][:], d[:])
            w1 = wpool.tile([128, 2 * OUT], MM_DT)
            nc.scalar.dma_start(w1[:], w1_d[:])

            # PE pre-warm on zeroed scratch while the first DMAs fly.
            # memset rides the (otherwise idle) vector queue so the SWDGE
            # queue's first instruction is seg1's DMA push.
            warm_x = wpool.tile([KT, CHUNK], MM_DT)
            nc.vector.memset(warm_x[:], 0.0)
            warm_ps = op.tile([128, CHUNK], f32, tag="warm", bufs=1)
            for _ in range(N_WARMUP):
                nc.tensor.matmul(
                    warm_ps[:], warm_x[:, 0:128], warm_x[:],
                    start=True, stop=True)

            o_tail = opool.tile([OUT, tail_w], f32, tag="osb_tail", bufs=1)

            def emit_layer2(g, w, c0, h_tiles):
                # layer 2: outT[10, seg], 2 accumulating matmuls
                o_ps = op.tile([OUT, w], f32, tag="ops", name=f"ops_{g}")
                nc.tensor.matmul(
                    o_ps[:], w1[0:128, 0:OUT], h_tiles[0][:],
                    start=True, stop=False)
                nc.tensor.matmul(
                    o_ps[:], w1[0:HID - 128, OUT:2 * OUT], h_tiles[1][:],
                    start=False, stop=True)
                # bias-add on the scalar engine: runs in PARALLEL with the
                # DVE's relu of the next segment (putting the tail ones on
                # DVE serialized behind its relu FIFO and measured worse).
                # All stores ride the ACT ring, directly behind the ident
                # that produced them; the last N_TAIL_MERGE segments write
                # slices of one SBUF tile and share one store so the
                # final push is a single transfer on the already-hot ring
                # (the SP ring has been idle for ~10us by then and a cold
                # ring adds ~1.5us completion latency).
                if g >= tail0:
                    lo = c0 - tail_c0
                    nc.scalar.activation(
                        o_tail[:, lo:lo + w], o_ps[:],
                        mybir.ActivationFunctionType.Identity,
                        bias=bia[0:OUT, 2:3])
                    if g == n_segs - 1:
                        nc.scalar.dma_start(
                            out_d[:, tail_c0:SHARD], o_tail[:])
                else:
                    o_sb = opool.tile([OUT, w], f32, tag="osb",
                                      name=f"osb_{g}")
                    nc.scalar.activation(
                        o_sb[:], o_ps[:],
                        mybir.ActivationFunctionType.Identity,
                        bias=bia[0:OUT, 2:3])
                    nc.scalar.dma_start(out_d[:, c0:c0 + w], o_sb[:])

            c0 = 0
            x_dmas = []
            pending = None   # layer 2 runs one segment behind layer 1,
            # so the PE never waits on the DVE relu at a seg boundary
            for g, w in enumerate(SEGS):
                xg = xpool.tile([KT, NKT, w], MM_DT, tag="xg",
                                name=f"xg_{g}")
                # even segments ride the SP HWDGE ring, odd the SWDGE
                # ring: both rings pull concurrently, so seg1 lands right
                # behind seg0 instead of round-robining against it (which
                # cost a ~1us PE stall at the first seg boundary)
                eng = nc.sync if g % 2 == 0 else nc.gpsimd
                dma = eng.dma_start(xg[:], xg_d[g][:])
                # one transfer in flight per ring: the SDMA engines
                # round-robin across queued transfers on a ring, which
                # delays the completion of the segment the PE needs next
                if g >= 2:
                    tile.add_dep_helper(
                        dma.ins, x_dmas[g - 2].ins, sync=True,
                        reason="chain same-ring x DMAs back-to-back")
                x_dmas.append(dma)

                # layer 1: hT[m0:m0+dm, seg], 7 accumulating matmuls per
                # m-tile; the m0 tile only needs the w0a image so its
                # matmuls can start while w0b is still in flight
                h_tiles = []
                for mi, (m0, dm) in enumerate(M_TILES):
                    h_ps = pp.tile([dm, w], f32, tag=f"hps{mi}",
                                   name=f"hps_{g}_{mi}")
                    for a in range(NKT):
                        nc.tensor.matmul(
                            h_ps[:],
                            w0t[mi][:, a, :],
                            xg[:, a, :],
                            start=(a == 0),
                            stop=(a == NKT - 1),
                        )
                    h_sb = hpool.tile([dm, w], MM_DT, tag=f"h{mi}",
                                      name=f"h_{g}_{mi}")
                    # fused bias + relu on the vector engine
                    nc.vector.tensor_scalar(
                        h_sb[:], h_ps[:], bia[0:dm, mi:mi + 1], 0.0,
                        add, amax)
                    h_tiles.append(h_sb)

                if pending is not None:
                    emit_layer2(*pending)
                pending = (g, w, c0, h_tiles)
                c0 += w

            emit_layer2(*pending)

    nc.compile()
    return nc


_program_cache = {}


def _get_program():
    key = (MM_DT, tuple(SEGS), N_WARMUP)
    if key not in _program_cache:
        _program_cache[key] = build_program()
    return _program_cache[key]


def kernel(**inputs: np.ndarray) -> np.ndarray:
    x = np.asarray(inputs["x"], dtype=np.float32)
    conv_w = np.asarray(inputs["conv_w"], dtype=np.float32)
    w0 = np.asarray(inputs["w0"], dtype=np.float32)
    b0 = np.asarray(inputs["b0"], dtype=np.float32)
    w1 = np.asarray(inputs["w1"], dtype=np.float32)
    b1 = np.asarray(inputs["b1"], dtype=np.float32)

    mm_np = _np_mm_dtype()
    w_eff = fold_conv_into_fc(conv_w, w0)
    w0a, w0b, w1sb, biases = pack_weights(w_eff, w1, b0, b1, mm_np)

    in_maps = []
    for i in range(N_CORES):
        xgs = pack_shard(x[i * SHARD:(i + 1) * SHARD], mm_np)
        m = {f"xg{g}": xg for g, xg in enumerate(xgs)}
        m.update({"w0a": w0a, "w0b": w0b, "w1sb": w1sb, "biases": biases})
        in_maps.append(m)

    nc = _get_program()

    profile = os.environ.get("BASS_KERNEL_PROFILE", "0") == "1"
    kwargs = {}
    if profile:
        _install_ntff_hook()
        kwargs = dict(trace=True, tmpdir=os.environ.get("BASS_KERNEL_TRACE_DIR"))
    try:
        res = run_bass_kernel_spmd(
            nc, in_maps, core_ids=list(range(N_CORES)), **kwargs)
    except Exception:
        # a previous process can leave a NeuronCore momentarily
        # unrecoverable (NRT_EXEC_UNIT_UNRECOVERABLE); one retry suffices
        import time
        time.sleep(5)
        res = run_bass_kernel_spmd(
            nc, in_maps, core_ids=list(range(N_CORES)), **kwargs)

    global last_exec_time_ns
    last_exec_time_ns = res.exec_time_ns

    out = np.empty((B, OUT), dtype=np.float32)
    for i in range(N_CORES):
        out[i * SHARD:(i + 1) * SHARD] = res.results[i]["out"].T
    return out



# revision 16
# speedup vs baseline: 1.1283x; 1.0498x over previous
"""Trainium2 Bass kernel for DigitConvolutionalModel forward pass.

Model: x[B,784] -> 3x3 valid conv (28x28 -> 26x26) -> flatten[676]
       -> Linear(676->200) + ReLU -> Linear(200->10).

Key algebraic optimization: the conv is linear and feeds straight into the
first Linear, so both fold into a single effective weight
W_eff[200,784] = w0 compose conv  (computed once on host, ~1.2 MFLOP).
The device then runs two dense GEMMs per batch shard:
    h = relu(x @ W_eff.T + b0);  out = h @ w1.T + b1

Sharding: pure data parallel over the batch dim across 8 NeuronCores
(4096 rows each); weights replicated; no collectives (forward only).

On-device layout is feature-major ("transposed") so the contraction dim
always lives on SBUF partitions: xT[784,n] -> hT[200,n] -> outT[10,n].
The host pre-packs x shards into exact SBUF tile images (k tiled 7x112)
so all x traffic is a handful of large single-ring DMAs whose partition
lines are multi-KB contiguous runs; group sizes are staggered (small
first) so compute starts early. Compute dtype bf16 (1 cyc/row matmuls,
half the DMA bytes); PSUM accumulates f32; bias+ReLU fused on the
vector engine; weights load on the ACT ring; output stores on SWDGE.
Dummy matmuls on zeroed scratch pre-warm the PE's HAM clock gate during
the first DMA's flight.
"""

import os
import sys
import types
import numpy as np

for _p in ("/opt/trn_rl_repo", "/root/.axon_site"):
    if os.path.isdir(_p) and _p not in sys.path:
        sys.path.insert(0, _p)

import concourse.bass as bass  # noqa: E402
import concourse.tile as tile  # noqa: E402
import concourse.mybir as mybir  # noqa: E402
from concourse import bacc  # noqa: E402
from concourse.bass_utils import run_bass_kernel_spmd  # noqa: E402

B = 32768
N_CORES = 8
SHARD = B // N_CORES          # 4096
KDIM = 784                    # 28*28 input features (conv folded in)
HID = 200
OUT = 10
CHUNK = 512                   # batch columns per matmul (moving free dim)
# batch-column widths per pipeline segment: narrow at the head (compute
# starts sooner, bridging the PE warm-up) and at the tail (shorter
# relu->fc2->store latency chain after the last big matmul)
SEGS = [256] + [512] * 7 + [256]
KT = 112                      # k-tile partition size (7 * 112 = 784)
NKT = KDIM // KT              # 7 k-tiles
M_TILES = [(0, 128), (128, 72)]  # hidden 200 = 128 + 72 PSUM partition tiles
N_WARMUP = 13                 # dummy matmuls to trip the HAM clock gate

# matmul operand dtype:
#   float32  — exact, 4 cyc/row           (~143us)
#   float32r — ~2-3 cyc/row, rel err 2e-4 (~71us)
#   bfloat16 — 1 cyc/row, half DMA bytes, rel err ~3e-3
MM_DT = mybir.dt.bfloat16

last_exec_time_ns = None      # set when BASS_KERNEL_PROFILE=1


def _install_ntff_hook():
    """Register the axon NTFF profile hook if the image's antenv lacks it."""
    try:
        from antenv.axon_hooks import get_axon_ntff_profile_hook  # noqa: F401
        return
    except ImportError:
        pass
    try:
        from trn_agent_boot.trn_boot import _ntff_profile_via_ctypes
        hook = _ntff_profile_via_ctypes("/opt/axon/libaxon_pjrt.so")
    except Exception:
        hook = None
    mod = types.ModuleType("antenv.axon_hooks")
    mod.get_axon_ntff_profile_hook = lambda: hook
    mod.set_axon_ntff_profile_hook = lambda h: None
    sys.modules["antenv.axon_hooks"] = mod


def _np_mm_dtype():
    if MM_DT == mybir.dt.bfloat16:
        import ml_dtypes
        return np.dtype(ml_dtypes.bfloat16)
    return np.dtype(np.float32)


def fold_conv_into_fc(conv_w: np.ndarray, w0: np.ndarray) -> np.ndarray:
    """W_eff[200,784] such that x @ W_eff.T == fc1(flatten(conv(x)))."""
    w0v = w0.reshape(HID, 26, 26).astype(np.float64)
    w_img = np.zeros((HID, 28, 28), dtype=np.float64)
    for ki in range(3):
        for kj in range(3):
            w_img[:, ki:ki + 26, kj:kj + 26] += w0v * np.float64(conv_w[ki, kj])
    return w_img.reshape(HID, KDIM).astype(np.float32)


def pack_shard(xs: np.ndarray, mm_np):
    """Pack one x shard [4096, 784] into per-group SBUF tile images.

    Group g (gsz chunks starting at chunk c0):
      xg[p, j, a, n] = x[(c0+j)*CHUNK + n, a*KT + p]
    Every SBUF partition line is one contiguous (j, a, n) run.
    """
    xsv = xs.reshape(SHARD, NKT, KT)
    arrays = []
    c0 = 0
    for w in SEGS:
        blk = xsv[c0:c0 + w]                        # [n, a, p]
        arrays.append(np.ascontiguousarray(
            blk.transpose(2, 1, 0).astype(mm_np)))  # [p, a, n]
        c0 += w
    return arrays


def pack_weights(w_eff: np.ndarray, w1: np.ndarray, b0, b1, mm_np):
    """Pack weights/biases into single-DMA SBUF images."""
    # w0sb[p, a, m] = W_eff[m, a*KT + p]
    w0sb = np.ascontiguousarray(
        w_eff.reshape(HID, NKT, KT).transpose(2, 1, 0).astype(mm_np))
    # w1sb[p, 0:10] = w1[:, p].T ; w1sb[0:72, 10:20] = w1[:, 128+p].T
    w1sb = np.zeros((128, 2 * OUT), dtype=mm_np)
    w1sb[:, :OUT] = w1[:, 0:128].T.astype(mm_np)
    w1sb[:HID - 128, OUT:] = w1[:, 128:HID].T.astype(mm_np)
    # bias[p, 0] = b0[p]; bias[0:72, 1] = b0[128:200]; bias[0:10, 2] = b1
    biases = np.zeros((128, 3), dtype=np.float32)
    biases[:, 0] = b0[0:128]
    biases[:HID - 128, 1] = b0[128:HID]
    biases[:OUT, 2] = b1
    return w0sb, w1sb, biases


def build_program():
    nc = bacc.Bacc("TRN2", target_bir_lowering=False, debug=False)
    f32 = mybir.dt.float32
    add = mybir.AluOpType.add
    amax = mybir.AluOpType.max

    xg_d = [
        nc.declare_dram_parameter(
            f"xg{g}", [KT, NKT, w], MM_DT, isOutput=False)
        for g, w in enumerate(SEGS)
    ]
    w0_d = nc.declare_dram_parameter("w0sb", [KT, NKT, HID], MM_DT, isOutput=False)
    w1_d = nc.declare_dram_parameter("w1sb", [128, 2 * OUT], MM_DT, isOutput=False)
    bia_d = nc.declare_dram_parameter("biases", [128, 3], f32, isOutput=False)
    out_d = nc.declare_dram_parameter("out", [OUT, SHARD], f32, isOutput=True)

    with tile.TileContext(nc) as tc:
        with (
            tc.tile_pool(name="weights", bufs=1) as wpool,
            tc.tile_pool(name="xin", bufs=3) as xpool,
            tc.tile_pool(name="hbuf", bufs=2) as hpool,
            tc.tile_pool(name="obuf", bufs=4) as opool,
            tc.tile_pool(name="psum", bufs=2, space=bass.MemorySpace.PSUM) as pp,
            tc.tile_pool(name="opsum", bufs=2, space=bass.MemorySpace.PSUM) as op,
        ):
            # weights + biases ride the ACT ring so the SP ring belongs
            # exclusively to the x stream (first-chunk completion time)
            w0 = wpool.tile([KT, NKT, HID], MM_DT)
            nc.scalar.dma_start(w0[:], w0_d[:])
            bia = wpool.tile([128, 3], f32)
            nc.scalar.dma_start(bia[:], bia_d[:])
            w1 = wpool.tile([128, 2 * OUT], MM_DT)
            nc.scalar.dma_start(w1[:], w1_d[:])

            # PE pre-warm on zeroed scratch while the first DMAs fly.
            # memset rides the (otherwise idle) vector queue so the SWDGE
            # queue's first instruction is seg1's DMA push.
            warm_x = wpool.tile([KT, CHUNK], MM_DT)
            nc.vector.memset(warm_x[:], 0.0)
            warm_ps = op.tile([128, CHUNK], f32, tag="warm", bufs=1)
            for _ in range(N_WARMUP):
                nc.tensor.matmul(
                    warm_ps[:], warm_x[:, 0:128], warm_x[:],
                    start=True, stop=True)

            def emit_layer2(g, w, c0, h_tiles):
                # layer 2: outT[10, seg], 2 accumulating matmuls
                o_ps = op.tile([OUT, w], f32, tag="ops", name=f"ops_{g}")
                nc.tensor.matmul(
                    o_ps[:], w1[0:128, 0:OUT], h_tiles[0][:],
                    start=True, stop=False)
                nc.tensor.matmul(
                    o_ps[:], w1[0:HID - 128, OUT:2 * OUT], h_tiles[1][:],
                    start=False, stop=True)
                o_sb = opool.tile([OUT, w], f32, tag="osb", name=f"osb_{g}")
                # bias-add on the scalar engine: runs in PARALLEL with the
                # DVE's relu of the next segment (putting the tail ones on
                # DVE serialized behind its relu FIFO and measured worse)
                nc.scalar.activation(
                    o_sb[:], o_ps[:],
                    mybir.ActivationFunctionType.Identity,
                    bias=bia[0:OUT, 2:3])
                # every output store rides the ACT ring, directly behind
                # the ident that produced o_sb — including the final one:
                # the ring is still hot from the previous store, while a
                # cold ring costs ~1.7us of completion latency
                nc.scalar.dma_start(out_d[:, c0:c0 + w], o_sb[:])

            c0 = 0
            x_dmas = []
            pending = None   # layer 2 runs one segment behind layer 1,
            # so the PE never waits on the DVE relu at a seg boundary
            for g, w in enumerate(SEGS):
                xg = xpool.tile([KT, NKT, w], MM_DT, tag="xg",
                                name=f"xg_{g}")
                # even segments ride the SP HWDGE ring, odd the SWDGE
                # ring: both rings pull concurrently, so seg1 lands right
                # behind seg0 instead of round-robining against it (which
                # cost a ~1us PE stall at the first seg boundary)
                eng = nc.sync if g % 2 == 0 else nc.gpsimd
                dma = eng.dma_start(xg[:], xg_d[g][:])
                # one transfer in flight per ring: the SDMA engines
                # round-robin across queued transfers on a ring, which
                # delays the completion of the segment the PE needs next
                if g >= 2:
                    tile.add_dep_helper(
                        dma.ins, x_dmas[g - 2].ins, sync=True,
                        reason="chain same-ring x DMAs back-to-back")
                x_dmas.append(dma)

                # layer 1: hT[m0:m0+dm, seg], 7 accumulating matmuls
                h_tiles = []
                for mi, (m0, dm) in enumerate(M_TILES):
                    h_ps = pp.tile([dm, w], f32, tag=f"hps{mi}",
                                   name=f"hps_{g}_{mi}")
                    for a in range(NKT):
                        nc.tensor.matmul(
                            h_ps[:],
                            w0[:, a, m0:m0 + dm],
                            xg[:, a, :],
                            start=(a == 0),
                            stop=(a == NKT - 1),
                        )
                    h_sb = hpool.tile([dm, w], MM_DT, tag=f"h{mi}",
                                      name=f"h_{g}_{mi}")
                    # fused bias + relu on the vector engine
                    nc.vector.tensor_scalar(
                        h_sb[:], h_ps[:], bia[0:dm, mi:mi + 1], 0.0,
                        add, amax)
                    h_tiles.append(h_sb)

                if pending is not None:
                    emit_layer2(*pending)
                pending = (g, w, c0, h_tiles)
                c0 += w

            emit_layer2(*pending)

    nc.compile()
    return nc


_program_cache = {}


def _get_program():
    key = (MM_DT, tuple(SEGS), N_WARMUP)
    if key not in _program_cache:
        _program_cache[key] = build_program()
    return _program_cache[key]


def kernel(**inputs: np.ndarray) -> np.ndarray:
    x = np.asarray(inputs["x"], dtype=np.float32)
    conv_w = np.asarray(inputs["conv_w"], dtype=np.float32)
    w0 = np.asarray(inputs["w0"], dtype=np.float32)
    b0 = np.asarray(inputs["b0"], dtype=np.float32)
    w1 = np.asarray(inputs["w1"], dtype=np.float32)
    b1 = np.asarray(inputs["b1"], dtype=np.float32)

    mm_np = _np_mm_dtype()
    w_eff = fold_conv_into_fc(conv_w, w0)
    w0sb, w1sb, biases = pack_weights(w_eff, w1, b0, b1, mm_np)

    in_maps = []
    for i in range(N_CORES):
        xgs = pack_shard(x[i * SHARD:(i + 1) * SHARD], mm_np)
        m = {f"xg{g}": xg for g, xg in enumerate(xgs)}
        m.update({"w0sb": w0sb, "w1sb": w1sb, "biases": biases})
        in_maps.append(m)

    nc = _get_program()

    profile = os.environ.get("BASS_KERNEL_PROFILE", "0") == "1"
    kwargs = {}
    if profile:
        _install_ntff_hook()
        kwargs = dict(trace=True, tmpdir=os.environ.get("BASS_KERNEL_TRACE_DIR"))
    try:
        res = run_bass_kernel_spmd(
            nc, in_maps, core_ids=list(range(N_CORES)), **kwargs)
    except Exception:
        # a previous process can leave a NeuronCore momentarily
        # unrecoverable (NRT_EXEC_UNIT_UNRECOVERABLE); one retry suffices
        import time
        time.sleep(5)
        res = run_bass_kernel_spmd(
            nc, in_maps, core_ids=list(range(N_CORES)), **kwargs)

    global last_exec_time_ns
    last_exec_time_ns = res.exec_time_ns

    out = np.empty((B, OUT), dtype=np.float32)
    for i in range(N_CORES):
        out[i * SHARD:(i + 1) * SHARD] = res.results[i]["out"].T
    return out

